# revision 27
# baseline (speedup 1.0000x reference)
"""Butterfly block-sparse linear kernel for Trainium2 (8 NeuronCores, SPMD).

Computes: y = blockdiag_butterfly(x, factorL, factorR) + bias
  x:(4,2048,4096) f32, factorL/factorR:(8,512,512) f32, bias:(4096,) f32

Math (reference):
  out1[b,k,q] = sum_p x[b, 512k+p] * factorL[k,q,p]      (8 blocks of 512x512)
  z[b,l,r]    = out1_flat[b, 8r+l]                        (butterfly permute)
  out2[b,l,s] = sum_r z[b,l,r] * factorR[l,s,r]
  y[b, 8s+l]  = out2[b,l,s] + bias[8s+l]

Strategy: data-parallel over the 8192 tokens (1024 tokens/core), factors
replicated. All on-chip data is bf16 (f32 PSUM accumulation, f32 bias),
halving HBM traffic vs f32 so the kernel is tensor-bound. Activations are
feature-major (features on SBUF partitions, tokens on the free axis).

The butterfly permute: host reorders factorL's output channels
q -> q' = 64*(q%8) + q//8, so stage-1 PSUM tile (k,qc) holds rows for
stage-2 blocks l=2qc (partitions h..h+64 for k even at h=0) and l=2qc+1.
The half already on the right partitions ("aligned", l-parity == k-parity)
is engine-copied straight into the z tile; the crossed half is staged and
partition-remapping SBUF->SBUF DMAs per (batch,k) move all 4 qc's.

Schedule: S1(b0) -> S1(b1) -> S2(b0) -> S2(b1). The z-permute barrier of
batch b is hidden under the ~30us of matmuls of the next phase, so the PE
never stalls and stays at its top DVFS state. Evictions alternate between
the DVE and ACT engines so neither becomes the pacing engine; stage-2
eviction fuses the per-partition bias and the bf16 downcast.

Output leaves the device in a device-friendly order (rows b,l,sc,ss); the
host does the final (cheap) gather back to token-major f32.
"""

import os
import numpy as np
from contextlib import ExitStack

NCORES = 8
TOK = 8192
TPC = TOK // NCORES          # tokens per core
T = 512                      # tokens per on-chip batch
NB = TPC // T                # 2 batches

_CACHE = {}
LAST_RESULT = None


def _build_program():
    import concourse.bacc as bacc
    import concourse.tile as tile
    import concourse.mybir as mybir

    F32 = mybir.dt.float32
    BF16 = mybir.dt.bfloat16

    nc = bacc.Bacc("TRN2", target_bir_lowering=False, debug=False)
    # x rows = (b, k, pp), cols = (pc, t): per (b,k) one [128,2048] tile,
    # 4KB contiguous per partition line.
    x = nc.dram_tensor("x", [NB * 8 * 128, 2048], BF16, kind="ExternalInput").ap()
    w1 = nc.dram_tensor("w1", [128, 16384], BF16, kind="ExternalInput").ap()
    w2 = nc.dram_tensor("w2", [128, 16384], BF16, kind="ExternalInput").ap()
    bias = nc.dram_tensor("bias", [128, 32], F32, kind="ExternalInput").ap()
    # out rows = (b, l, sc, ss), cols = t (device order; host unscrambles)
    out = nc.dram_tensor("out", [NB * 4096, T], BF16, kind="ExternalOutput").ap()

    x_r = x.rearrange("(b k p) c -> b k p c", b=NB, k=8)
    out_r = out.rearrange("(g a p) t -> g p a t", a=4, p=128)

    with tile.TileContext(nc) as tc, ExitStack() as ctx:
        wpool = ctx.enter_context(tc.tile_pool(name="w", bufs=1))
        xpool = ctx.enter_context(tc.tile_pool(name="x", bufs=10))
        spool = ctx.enter_context(tc.tile_pool(name="stg", bufs=8))
        zpool = ctx.enter_context(tc.tile_pool(name="z", bufs=1))
        opool = ctx.enter_context(tc.tile_pool(name="o", bufs=3))
        ps1 = ctx.enter_context(tc.tile_pool(name="ps1", bufs=4, space="PSUM"))
        ps2 = ctx.enter_context(tc.tile_pool(name="ps2", bufs=4, space="PSUM"))

        bt = wpool.tile([128, 32], F32, tag="bias")
        w1t = wpool.tile([128, 16384], BF16, tag="w1")
        w2t = wpool.tile([128, 16384], BF16, tag="w2")
        zts = [
            zpool.tile([128, NB * 8 * T], BF16, name=f"z_{c}", tag=f"z_{c}")
            for c in range(4)
        ]

        def _evcopy(eng, dst, src):
            if eng is nc.scalar:
                eng.activation(dst, src, mybir.ActivationFunctionType.Identity)
            else:
                eng.tensor_copy(dst, src)

        xloads = {}

        def load_x(b, k, split=False):
            # k-parity picks the queue so each queue streams at half rate
            eng = nc.sync if k % 2 == 0 else nc.scalar
            eng2 = nc.scalar if k % 2 == 0 else nc.sync
            xt = xpool.tile([128, 2048], BF16, tag="xt")
            if split:
                eng.dma_start(xt[:, 0:1024], x_r[b, k, :, 0:1024])
                eng2.dma_start(xt[:, 1024:2048], x_r[b, k, :, 1024:2048])
            else:
                eng.dma_start(xt[:], x_r[b, k])
            xloads[(b, k)] = xt

        def load_w(wt, src, j, split=False):
            if split:
                nc.gpsimd.dma_start(
                    wt[:, j * 2048 : j * 2048 + 1024],
                    src[:, j * 2048 : j * 2048 + 1024],
                )
                nc.gpsimd.dma_start(
                    wt[:, j * 2048 + 1024 : (j + 1) * 2048],
                    src[:, j * 2048 + 1024 : (j + 1) * 2048],
                )
            else:
                nc.gpsimd.dma_start(
                    wt[:, j * 2048 : (j + 1) * 2048],
                    src[:, j * 2048 : (j + 1) * 2048],
                )

        def s1(b, k):
            xt = xloads.pop((b, k))
            c, h = k // 2, 64 * (k % 2)
            hx = 64 - h
            zv = zts[c].rearrange("p (b l t) -> p b l t", b=NB, l=8)
            stg = spool.tile([128, 2 * T], BF16, tag="stg")
            for qc in range(4):
                p1 = ps1.tile([128, T], F32, tag="p1")
                for pc in range(4):
                    col = k * 2048 + (pc * 4 + qc) * 128
                    nc.tensor.matmul(
                        p1[:],
                        w1t[:, col : col + 128],
                        xt[:, pc * T : (pc + 1) * T],
                        start=(pc == 0),
                        stop=(pc == 3),
                    )
                l_a = 2 * qc + (k % 2)
                sh = 64 * (qc % 2)
                e1 = nc.vector if qc % 2 == 0 else nc.scalar
                e2 = nc.scalar if qc % 2 == 0 else nc.vector
                _evcopy(e1, zv[h : h + 64, b, l_a, :], p1[h : h + 64, :])
                _evcopy(
                    e2,
                    stg[sh : sh + 64, (qc // 2) * T : (qc // 2 + 1) * T],
                    p1[hx : hx + 64, :],
                )
            # crossed-half DMAs per (b,k): stg[a*64+p, c*T+t] holds the
            # crossed half of qc = 2c+a, destined for l-slot 2qc + (1-k%2);
            # one 3-dim DMA per staging partition half (a = qc%2)
            par = 1 - k % 2
            qd1, qd2 = (nc.sync, nc.scalar) if k % 2 == 0 else (nc.scalar, nc.sync)
            qd1.dma_start(
                zv[h : h + 64, b, par::4, :],
                stg[0:64, :].rearrange("p (c t) -> p c t", c=2),
            )
            qd2.dma_start(
                zv[h : h + 64, b, par + 2 :: 4, :],
                stg[64:128, :].rearrange("p (c t) -> p c t", c=2),
            )

        def s2(b, l):
            ot = opool.tile([128, 4 * T], BF16, tag="ot")
            for sc in range(4):
                p2 = ps2.tile([128, T], F32, tag="p2")
                for c in range(4):
                    col = l * 2048 + (c * 4 + sc) * 128
                    nc.tensor.matmul(
                        p2[:],
                        w2t[:, col : col + 128],
                        zts[c][:, (b * 8 + l) * T : (b * 8 + l + 1) * T],
                        start=(c == 0),
                        stop=(c == 3),
                    )
                bcol = bt[:, l * 4 + sc : l * 4 + sc + 1]
                if sc % 2 == 0:
                    nc.vector.tensor_scalar(
                        out=ot[:, sc * T : (sc + 1) * T],
                        in0=p2[:],
                        scalar1=bcol,
                        scalar2=None,
                        op0=mybir.AluOpType.add,
                    )
                else:
                    nc.scalar.activation(
                        ot[:, sc * T : (sc + 1) * T],
                        p2[:],
                        mybir.ActivationFunctionType.Identity,
                        bias=bcol,
                    )
            # split the store in halves on two queues: finer overlap and an
            # earlier final-store start at the kernel tail
            g = b * 8 + l
            nc.sync.dma_start(
                out_r[g, :, 0:2, :],
                ot[:, 0 : 2 * T].rearrange("p (a t) -> p a t", a=2),
            )
            nc.scalar.dma_start(
                out_r[g, :, 2:4, :],
                ot[:, 2 * T : 4 * T].rearrange("p (a t) -> p a t", a=2),
            )

        # ---- schedule: S1(b0) S1(b1) S2(b0) S2(b1), loads front-run ----
        # first deps go on the HWDGE queues (gpsimd SW-DGE has a slow cold
        # start) in small chunks so the PE starts ASAP: the first matmul
        # needs only x(0,0) cols 0:512 and w1 cols 0:1024
        xt0 = xpool.tile([128, 2048], BF16, tag="xt")
        nc.sync.dma_start(xt0[:, 0:512], x_r[0, 0, :, 0:512])
        nc.scalar.dma_start(w1t[:, 0:1024], w1[:, 0:1024])
        nc.sync.dma_start(xt0[:, 512:2048], x_r[0, 0, :, 512:2048])
        nc.scalar.dma_start(w1t[:, 1024:2048], w1[:, 1024:2048])
        xloads[(0, 0)] = xt0
        LOOK = 3
        for j in range(1, LOOK):
            load_w(w1t, w1, j)
            load_x(0, j)
        nc.gpsimd.dma_start(bt[:], bias[:])
        for k in range(8):
            if k + LOOK < 8:
                load_w(w1t, w1, k + LOOK)
                load_x(0, k + LOOK)
            s1(0, k)
            if 3 <= k < 7:
                load_x(1, 2 * (k - 3))
                load_x(1, 2 * (k - 3) + 1)
        for k in range(8):
            load_w(w2t, w2, k)
            s1(1, k)
        for l in range(8):
            s2(0, l)
        for l in range(8):
            s2(1, l)
    nc.compile()
    return nc


def _get_program():
    if "nc" not in _CACHE:
        _CACHE["nc"] = _build_program()
    return _CACHE["nc"]


def _ensure_ntff_hook():
    """Bridge the axon NTFF profile hook when the image's antenv lacks it."""
    import sys, types

    try:
        from antenv.axon_hooks import get_axon_ntff_profile_hook  # noqa: F401

        return
    except ImportError:
        pass
    try:
        from trn_agent_boot.trn_boot import _ntff_profile_via_ctypes

        hook = _ntff_profile_via_ctypes("/opt/axon/libaxon_pjrt.so")
        mod = types.ModuleType("antenv.axon_hooks")
        _h = {"hook": hook}
        mod.set_axon_ntff_profile_hook = lambda h: _h.__setitem__("hook", h)
        mod.get_axon_ntff_profile_hook = lambda: _h["hook"]
        sys.modules["antenv.axon_hooks"] = mod
        import antenv

        antenv.axon_hooks = mod
    except Exception:
        pass


def kernel(x, factorL, factorR, bias):
    global LAST_RESULT
    import ml_dtypes
    from concourse.bass_utils import run_bass_kernel_spmd

    BF = ml_dtypes.bfloat16
    x = np.asarray(x, dtype=np.float32)
    factorL = np.asarray(factorL, dtype=np.float32)
    factorR = np.asarray(factorR, dtype=np.float32)
    bias = np.asarray(bias, dtype=np.float32)

    # host-side marshalling (not device-timed)
    xt = np.ascontiguousarray(x.reshape(TOK, 4096).T).astype(BF)  # (4096, 8192)
    qp = np.arange(512)
    q_of_qprime = 8 * (qp % 64) + qp // 64
    w1p = factorL.transpose(0, 2, 1)[:, :, q_of_qprime]  # (8, p, q')
    w1dev = np.ascontiguousarray(
        w1p.reshape(8, 4, 128, 4, 128).transpose(2, 0, 1, 3, 4).reshape(128, 16384)
    ).astype(BF)
    w2p = factorR.transpose(0, 2, 1)  # (8, r, s)
    w2dev = np.ascontiguousarray(
        w2p.reshape(8, 4, 128, 4, 128).transpose(2, 0, 1, 3, 4).reshape(128, 16384)
    ).astype(BF)
    biasdev = np.ascontiguousarray(
        bias.reshape(4, 128, 8).transpose(1, 2, 0).reshape(128, 32)
    )

    in_maps = []
    for c in range(NCORES):
        xc = xt[:, c * TPC : (c + 1) * TPC]  # (4096 feat, 1024 tok) bf16
        # rows (k,pc,pp) cols (b,t) -> [(b k pp), (pc t)]
        xdev = np.ascontiguousarray(
            xc.reshape(8, 4, 128, NB, T)
            .transpose(3, 0, 2, 1, 4)
            .reshape(NB * 8 * 128, 2048)
        )
        in_maps.append({"x": xdev, "w1": w1dev, "w2": w2dev, "bias": biasdev})

    nc = _get_program()
    trace = os.environ.get("BUTTERFLY_TRACE", "0") == "1"
    if trace:
        _ensure_ntff_hook()
    LAST_RESULT = run_bass_kernel_spmd(
        nc, in_maps, list(range(NCORES)), trace=trace
    )
    # device out rows = (b, l, sc, ss), cols = t  ->  (tok, feat j=8s+l)
    parts = []
    for c in range(NCORES):
        o = np.asarray(LAST_RESULT.results[c]["out"]).astype(np.float32)
        y = o.reshape(NB, 8, 4, 128, T).transpose(0, 4, 2, 3, 1).reshape(TPC, 4096)
        parts.append(y)
    return np.concatenate(parts, axis=0).reshape(4, 2048, 4096)


# revision 28
# speedup vs baseline: 1.0530x; 1.0530x over previous
"""Butterfly block-sparse linear kernel for Trainium2 (8 NeuronCores, SPMD).

Computes: y = blockdiag_butterfly(x, factorL, factorR) + bias
  x:(4,2048,4096) f32, factorL/factorR:(8,512,512) f32, bias:(4096,) f32

Math (reference):
  out1[b,k,q] = sum_p x[b, 512k+p] * factorL[k,q,p]      (8 blocks of 512x512)
  z[b,l,r]    = out1_flat[b, 8r+l]                        (butterfly permute)
  out2[b,l,s] = sum_r z[b,l,r] * factorR[l,s,r]
  y[b, 8s+l]  = out2[b,l,s] + bias[8s+l]

Strategy: data-parallel over the 8192 tokens (1024 tokens/core), factors
replicated. All on-chip data is bf16 (f32 PSUM accumulation, f32 bias),
halving HBM traffic vs f32 so the kernel is tensor-bound. Activations are
feature-major (features on SBUF partitions, tokens on the free axis).

The butterfly permute: host reorders factorL's output channels
q -> q' = 64*(q%8) + q//8, so stage-1 PSUM tile (k,qc) holds rows for
stage-2 blocks l=2qc (partitions h..h+64 for k even at h=0) and l=2qc+1.
The half already on the right partitions ("aligned", l-parity == k-parity)
is engine-copied straight into the z tile; the crossed half is staged and
partition-remapping SBUF->SBUF DMAs per (batch,k) move all 4 qc's.

Schedule: S1(b0) -> S1(b1) -> S2(b0) -> S2(b1). The z-permute barrier of
batch b is hidden under the ~30us of matmuls of the next phase, so the PE
never stalls and stays at its top DVFS state. Evictions alternate between
the DVE and ACT engines so neither becomes the pacing engine; stage-2
eviction fuses the per-partition bias and the bf16 downcast.

Output leaves the device in a device-friendly order (rows b,l,sc,ss); the
host does the final (cheap) gather back to token-major f32.
"""

import os
import numpy as np
from contextlib import ExitStack

NCORES = 8
TOK = 8192
TPC = TOK // NCORES          # tokens per core
T = 512                      # tokens per on-chip batch
NB = TPC // T                # 2 batches

_CACHE = {}
LAST_RESULT = None


def _build_program():
    import concourse.bacc as bacc
    import concourse.tile as tile
    import concourse.mybir as mybir

    F32 = mybir.dt.float32
    BF16 = mybir.dt.bfloat16

    nc = bacc.Bacc("TRN2", target_bir_lowering=False, debug=False)
    # x rows = (b, k, pp), cols = (pc, t): per (b,k) one [128,2048] tile,
    # 4KB contiguous per partition line.
    x = nc.dram_tensor("x", [NB * 8 * 128, 2048], BF16, kind="ExternalInput").ap()
    w1 = nc.dram_tensor("w1", [128, 16384], BF16, kind="ExternalInput").ap()
    w2 = nc.dram_tensor("w2", [128, 16384], BF16, kind="ExternalInput").ap()
    bias = nc.dram_tensor("bias", [128, 32], F32, kind="ExternalInput").ap()
    # out rows = (b, l, sc, ss), cols = t (device order; host unscrambles)
    out = nc.dram_tensor("out", [NB * 4096, T], BF16, kind="ExternalOutput").ap()

    x_r = x.rearrange("(b k p) c -> b k p c", b=NB, k=8)
    out_r = out.rearrange("(g a p) t -> g p a t", a=4, p=128)

    with tile.TileContext(nc) as tc, ExitStack() as ctx:
        wpool = ctx.enter_context(tc.tile_pool(name="w", bufs=1))
        xpool = ctx.enter_context(tc.tile_pool(name="x", bufs=10))
        spool = ctx.enter_context(tc.tile_pool(name="stg", bufs=8))
        zpool = ctx.enter_context(tc.tile_pool(name="z", bufs=1))
        opool = ctx.enter_context(tc.tile_pool(name="o", bufs=3))
        ps1 = ctx.enter_context(tc.tile_pool(name="ps1", bufs=4, space="PSUM"))
        ps2 = ctx.enter_context(tc.tile_pool(name="ps2", bufs=4, space="PSUM"))

        bt = wpool.tile([128, 32], F32, tag="bias")
        w1t = wpool.tile([128, 16384], BF16, tag="w1")
        w2t = wpool.tile([128, 16384], BF16, tag="w2")
        zts = [
            zpool.tile([128, NB * 8 * T], BF16, name=f"z_{c}", tag=f"z_{c}")
            for c in range(4)
        ]

        def _evcopy(eng, dst, src):
            if eng is nc.scalar:
                eng.activation(dst, src, mybir.ActivationFunctionType.Identity)
            else:
                eng.tensor_copy(dst, src)

        xloads = {}

        def load_x(b, k, split=False):
            # k-parity picks the queue so each queue streams at half rate
            eng = nc.sync if k % 2 == 0 else nc.scalar
            eng2 = nc.scalar if k % 2 == 0 else nc.sync
            xt = xpool.tile([128, 2048], BF16, tag="xt")
            if split:
                eng.dma_start(xt[:, 0:1024], x_r[b, k, :, 0:1024])
                eng2.dma_start(xt[:, 1024:2048], x_r[b, k, :, 1024:2048])
            else:
                eng.dma_start(xt[:], x_r[b, k])
            xloads[(b, k)] = xt

        def load_w(wt, src, j, split=False):
            if split:
                nc.gpsimd.dma_start(
                    wt[:, j * 2048 : j * 2048 + 1024],
                    src[:, j * 2048 : j * 2048 + 1024],
                )
                nc.gpsimd.dma_start(
                    wt[:, j * 2048 + 1024 : (j + 1) * 2048],
                    src[:, j * 2048 + 1024 : (j + 1) * 2048],
                )
            else:
                nc.gpsimd.dma_start(
                    wt[:, j * 2048 : (j + 1) * 2048],
                    src[:, j * 2048 : (j + 1) * 2048],
                )

        def s1(b, k):
            xt = xloads.pop((b, k))
            c, h = k // 2, 64 * (k % 2)
            hx = 64 - h
            zv = zts[c].rearrange("p (b l t) -> p b l t", b=NB, l=8)
            stg = spool.tile([128, 2 * T], BF16, tag="stg")
            for qc in range(4):
                p1 = ps1.tile([128, T], F32, tag="p1")
                for pc in range(4):
                    col = k * 2048 + (pc * 4 + qc) * 128
                    nc.tensor.matmul(
                        p1[:],
                        w1t[:, col : col + 128],
                        xt[:, pc * T : (pc + 1) * T],
                        start=(pc == 0),
                        stop=(pc == 3),
                    )
                l_a = 2 * qc + (k % 2)
                sh = 64 * (qc % 2)
                e1 = nc.vector if qc % 2 == 0 else nc.scalar
                e2 = nc.scalar if qc % 2 == 0 else nc.vector
                _evcopy(e1, zv[h : h + 64, b, l_a, :], p1[h : h + 64, :])
                _evcopy(
                    e2,
                    stg[sh : sh + 64, (qc // 2) * T : (qc // 2 + 1) * T],
                    p1[hx : hx + 64, :],
                )
            # crossed-half DMAs per (b,k): stg[a*64+p, c*T+t] holds the
            # crossed half of qc = 2c+a, destined for l-slot 2qc + (1-k%2);
            # one 3-dim DMA per staging partition half (a = qc%2)
            par = 1 - k % 2
            qd1, qd2 = (nc.sync, nc.scalar) if k % 2 == 0 else (nc.scalar, nc.sync)
            qd1.dma_start(
                zv[h : h + 64, b, par::4, :],
                stg[0:64, :].rearrange("p (c t) -> p c t", c=2),
            )
            qd2.dma_start(
                zv[h : h + 64, b, par + 2 :: 4, :],
                stg[64:128, :].rearrange("p (c t) -> p c t", c=2),
            )

        def s2(b, l):
            ot = opool.tile([128, 4 * T], BF16, tag="ot")
            for sc in range(4):
                p2 = ps2.tile([128, T], F32, tag="p2")
                for c in range(4):
                    col = l * 2048 + (c * 4 + sc) * 128
                    nc.tensor.matmul(
                        p2[:],
                        w2t[:, col : col + 128],
                        zts[c][:, (b * 8 + l) * T : (b * 8 + l + 1) * T],
                        start=(c == 0),
                        stop=(c == 3),
                    )
                bcol = bt[:, l * 4 + sc : l * 4 + sc + 1]
                if sc % 2 == 0:
                    nc.vector.tensor_scalar(
                        out=ot[:, sc * T : (sc + 1) * T],
                        in0=p2[:],
                        scalar1=bcol,
                        scalar2=None,
                        op0=mybir.AluOpType.add,
                    )
                else:
                    nc.scalar.activation(
                        ot[:, sc * T : (sc + 1) * T],
                        p2[:],
                        mybir.ActivationFunctionType.Identity,
                        bias=bcol,
                    )
            # split the store in halves on two queues: finer overlap and an
            # earlier final-store start at the kernel tail
            g = b * 8 + l
            nc.sync.dma_start(
                out_r[g, :, 0:2, :],
                ot[:, 0 : 2 * T].rearrange("p (a t) -> p a t", a=2),
            )
            nc.scalar.dma_start(
                out_r[g, :, 2:4, :],
                ot[:, 2 * T : 4 * T].rearrange("p (a t) -> p a t", a=2),
            )

        # ---- schedule: S1(b0) S1(b1) S2(b0) S2(b1), loads front-run ----
        # first deps go on the HWDGE queues (gpsimd SW-DGE has a slow cold
        # start) in small chunks so the PE starts ASAP: the first matmul
        # needs only x(0,0) cols 0:512 and w1 cols 0:1024
        xt0 = xpool.tile([128, 2048], BF16, tag="xt")
        nc.sync.dma_start(xt0[:, 0:512], x_r[0, 0, :, 0:512])
        nc.scalar.dma_start(w1t[:, 0:1024], w1[:, 0:1024])
        nc.sync.dma_start(xt0[:, 512:2048], x_r[0, 0, :, 512:2048])
        nc.scalar.dma_start(w1t[:, 1024:2048], w1[:, 1024:2048])
        xloads[(0, 0)] = xt0
        LOOK = 3
        for j in range(1, LOOK):
            load_w(w1t, w1, j)
            load_x(0, j)
        nc.gpsimd.dma_start(bt[:], bias[:])
        for k in range(8):
            if k + LOOK < 8:
                load_w(w1t, w1, k + LOOK)
                load_x(0, k + LOOK)
            s1(0, k)
            if 3 <= k < 7:
                load_x(1, 2 * (k - 3))
                load_x(1, 2 * (k - 3) + 1)
        for k in range(8):
            load_w(w2t, w2, k)
            s1(1, k)
        with tc.tile_wait_until(0.055):
            for l in range(8):
                s2(0, l)
        with tc.tile_wait_until(0.085):
            for l in range(8):
                s2(1, l)
    nc.compile()
    return nc


def _get_program():
    if "nc" not in _CACHE:
        _CACHE["nc"] = _build_program()
    return _CACHE["nc"]


def _ensure_ntff_hook():
    """Bridge the axon NTFF profile hook when the image's antenv lacks it."""
    import sys, types

    try:
        from antenv.axon_hooks import get_axon_ntff_profile_hook  # noqa: F401

        return
    except ImportError:
        pass
    try:
        from trn_agent_boot.trn_boot import _ntff_profile_via_ctypes

        hook = _ntff_profile_via_ctypes("/opt/axon/libaxon_pjrt.so")
        mod = types.ModuleType("antenv.axon_hooks")
        _h = {"hook": hook}
        mod.set_axon_ntff_profile_hook = lambda h: _h.__setitem__("hook", h)
        mod.get_axon_ntff_profile_hook = lambda: _h["hook"]
        sys.modules["antenv.axon_hooks"] = mod
        import antenv

        antenv.axon_hooks = mod
    except Exception:
        pass


def kernel(x, factorL, factorR, bias):
    global LAST_RESULT
    import ml_dtypes
    from concourse.bass_utils import run_bass_kernel_spmd

    BF = ml_dtypes.bfloat16
    x = np.asarray(x, dtype=np.float32)
    factorL = np.asarray(factorL, dtype=np.float32)
    factorR = np.asarray(factorR, dtype=np.float32)
    bias = np.asarray(bias, dtype=np.float32)

    # host-side marshalling (not device-timed)
    xt = np.ascontiguousarray(x.reshape(TOK, 4096).T).astype(BF)  # (4096, 8192)
    qp = np.arange(512)
    q_of_qprime = 8 * (qp % 64) + qp // 64
    w1p = factorL.transpose(0, 2, 1)[:, :, q_of_qprime]  # (8, p, q')
    w1dev = np.ascontiguousarray(
        w1p.reshape(8, 4, 128, 4, 128).transpose(2, 0, 1, 3, 4).reshape(128, 16384)
    ).astype(BF)
    w2p = factorR.transpose(0, 2, 1)  # (8, r, s)
    w2dev = np.ascontiguousarray(
        w2p.reshape(8, 4, 128, 4, 128).transpose(2, 0, 1, 3, 4).reshape(128, 16384)
    ).astype(BF)
    biasdev = np.ascontiguousarray(
        bias.reshape(4, 128, 8).transpose(1, 2, 0).reshape(128, 32)
    )

    in_maps = []
    for c in range(NCORES):
        xc = xt[:, c * TPC : (c + 1) * TPC]  # (4096 feat, 1024 tok) bf16
        # rows (k,pc,pp) cols (b,t) -> [(b k pp), (pc t)]
        xdev = np.ascontiguousarray(
            xc.reshape(8, 4, 128, NB, T)
            .transpose(3, 0, 2, 1, 4)
            .reshape(NB * 8 * 128, 2048)
        )
        in_maps.append({"x": xdev, "w1": w1dev, "w2": w2dev, "bias": biasdev})

    nc = _get_program()
    trace = os.environ.get("BUTTERFLY_TRACE", "0") == "1"
    if trace:
        _ensure_ntff_hook()
    LAST_RESULT = run_bass_kernel_spmd(
        nc, in_maps, list(range(NCORES)), trace=trace
    )
    # device out rows = (b, l, sc, ss), cols = t  ->  (tok, feat j=8s+l)
    parts = []
    for c in range(NCORES):
        o = np.asarray(LAST_RESULT.results[c]["out"]).astype(np.float32)
        y = o.reshape(NB, 8, 4, 128, T).transpose(0, 4, 2, 3, 1).reshape(TPC, 4096)
        parts.append(y)
    return np.concatenate(parts, axis=0).reshape(4, 2048, 4096)
